# revision 71
# baseline (speedup 1.0000x reference)
"""GQA attention + RoPE, tensor-parallel across 8 NeuronCores (Bass/Tile).

Model: x(1,2048,2048) -> Q=xWq (32 heads x 64), K/V=xWk/xWv (8 kv heads),
RoPE on q/k, causal softmax attention (GQA: 4 q heads per kv head), out-proj.

Sharding: head-parallel. Core i gets q heads 4i..4i+3 (Wq cols), kv head i
(Wk/Wv cols), Wo rows 256i..256i+256. Each core computes a partial (2048,2048)
output; host sums the 8 partials (the "all-reduce").

v2 changes vs v1 (427us baseline):
  * All matmul operands bf16 (psum accum stays fp32): same PE rate as f32r
    but FWL halves LDWEIGHTS, DMA bytes halve, DVE gets 2x/4x modes.
  * RoPE without the ACT pre-copy: out = q*C + swap32(q*S2) with S2 a
    pre-swapped sign-adjusted sin table, so the DMA half-swap reads the
    DVE product directly (psum -> 2 DVE muls -> 4 small DMAs -> add).
  * Softmax normalize: reciprocal_approx_fast directly on the psum den row
    (was: 1-partition DMA + 3.3us DVE reciprocal), gpsimd broadcast from
    partition 64, single mul. No den DMA.
  * Causal handling: ctx matmuls restrict rhs/out to the live column range
    instead of zero-padding e (drops the zpad DMAs).
  * xc streamed as 4 DMAs of 4 k-blocks; emission order attn(c), proj(c+1),
    outproj(c) so ready proj/outproj matmuls fill PE stalls.
"""

import numpy as np
from contextlib import ExitStack

import concourse.bass as bass
from concourse import bacc
import concourse.tile as tile
from concourse import mybir
from concourse.bass_utils import run_bass_kernel_spmd

F32 = mybir.dt.float32
BF = mybir.dt.bfloat16
AF = mybir.ActivationFunctionType

S = 2048          # sequence length
D = 2048          # model dim
HD = 64           # head dim
NCORES = 8
QH = 4            # q heads per core
QC = QH * HD      # 256 q columns per core
SC = 512          # seq chunk width
NSC = S // SC     # 4 chunks
KB = D // 128     # 16 feature blocks
SCALE = 1.0 / 8.0  # 1/sqrt(64)

_NC = None


def _build():
    nc = bacc.Bacc(None)
    xT = nc.declare_dram_parameter("xT", [128, KB, S], BF, isOutput=False)
    wq = nc.declare_dram_parameter("wq", [128, KB, QC], BF, isOutput=False)
    wkv = nc.declare_dram_parameter("wkv", [128, KB, 128], BF, isOutput=False)
    wo = nc.declare_dram_parameter("wo", [128, 2, D], BF, isOutput=False)
    ctab = nc.declare_dram_parameter("ctab", [128, S], BF, isOutput=False)
    stab2 = nc.declare_dram_parameter("stab2", [128, S], BF, isOutput=False)
    trimask = nc.declare_dram_parameter("trimask", [128, 128], BF, isOutput=False)
    eye = nc.declare_dram_parameter("eye", [64, 64], F32, isOutput=False)
    vones = nc.declare_dram_parameter("vones", [128, KB, 2], BF, isOutput=False)
    vpad = nc.declare_dram_parameter("vpad", [128, KB, 63], BF, isOutput=False)
    out = nc.declare_dram_parameter("out", [S, D], BF, isOutput=True)

    with tile.TileContext(nc) as tc, ExitStack() as ctx:
        sb = ctx.enter_context(tc.tile_pool(name="sb", bufs=1))
        xp = ctx.enter_context(tc.tile_pool(name="xp", bufs=2))
        wk_ = ctx.enter_context(tc.tile_pool(name="wk", bufs=2))
        pp = ctx.enter_context(tc.tile_pool(name="pp", bufs=2, space="PSUM"))

        # ---- HAM warmup: ~115 dependency-free matmuls run from t=0 and
        # bridge the DMA-bound startup (~26us), so the real pipeline starts
        # at full PE clock instead of paying the 3.4us half-rate ramp. The
        # result is DMA'd to out[0:1] (overwritten later) to defeat DCE. ----
        ms = sb.tile([128, SC], BF)
        nc.vector.memset(ms, 0.25)
        dps = pp.tile([128, SC], F32, name="warm", tag="po")
        for i in range(115):
            nc.tensor.matmul(dps, lhsT=ms[:, 0:128], rhs=ms,
                             start=(i == 0), stop=(i == 114))
        wsc = wk_.tile([1, SC], BF, name="wsc", tag="wsc")
        nc.vector.tensor_copy(wsc, dps[0:1, :])
        nc.sync.dma_start(out=out[0:1, 0:SC], in_=wsc)

        # ---- persistent constants (ordered so chunk-0 Q matmuls + rope can
        # start ASAP; wo & attn-only tables load in the background) ----
        wq_sb = sb.tile([128, KB, QC], BF)
        nc.sync.dma_start(out=wq_sb[:, 0:4, :], in_=wq[:, 0:4, :])
        ctab_sb = sb.tile([128, S], BF)
        nc.sync.dma_start(out=ctab_sb, in_=ctab[:, :])
        stab_sb = sb.tile([128, S], BF)
        nc.sync.dma_start(out=stab_sb, in_=stab2[:, :])
        nc.sync.dma_start(out=wq_sb[:, 4:KB, :], in_=wq[:, 4:KB, :])
        wkv_sb = sb.tile([128, KB, 128], BF)
        nc.sync.dma_start(out=wkv_sb, in_=wkv[:, :, :])
        tri_sb = sb.tile([128, 128], BF)
        nc.gpsimd.dma_start(out=tri_sb, in_=trimask[:, :])
        eye_sb = sb.tile([64, 64], F32)
        nc.gpsimd.dma_start(out=eye_sb, in_=eye[:, :])
        wo_sb = sb.tile([128, 2, D], BF)
        nc.gpsimd.dma_start(out=wo_sb, in_=wo[:, :, :])

        # ---- persistent activations ----
        qt0 = sb.tile([128, S], BF)   # q^T heads 0,1 (roped)
        qt1 = sb.tile([128, S], BF)   # q^T heads 2,3
        qts = [qt0, qt1]
        kt_sb = sb.tile([128, S], BF)  # rows 0-63 k^T roped; 64-127 duplicate
        v_sb = sb.tile([128, KB, HD + 2], BF)  # [V | ones | pad] (for idx1)
        v_sb2 = sb.tile([128, KB, 128], BF)    # [ones | zeros*63 | V] (idx0)
        ct0 = sb.tile([128, S], BF)   # normalized ctx^T: rows 0-63 head 2u+1,
        ct1 = sb.tile([128, S], BF)   # rows 64-127 head 2u (wo host-reordered)
        cts = [ct0, ct1]
        nc.gpsimd.dma_start(out=v_sb[:, :, HD:HD + 2], in_=vones[:, :, :])
        nc.gpsimd.dma_start(out=v_sb2[:, :, 0:1], in_=vones[:, :, 0:1])
        nc.gpsimd.dma_start(out=v_sb2[:, :, 1:64], in_=vpad[:, :, :])

        def proj_gen(c):
            """Projection for chunk c as 3 interleavable units (yield after
            each): Q(u0)+rope-muls / Q(u1)+swap+adds / KV+k-rope+V."""
            cs = slice(c * SC, (c + 1) * SC)
            xc = xp.tile([128, KB, SC], BF, name=f"xc_{c}", tag="xc")
            for g in range(4):
                nc.sync.dma_start(out=xc[:, 4 * g:4 * g + 4, :],
                                  in_=xT[:, 4 * g:4 * g + 4, cs])
            # Q projection + rope, one 128-tile (2 heads) at a time; the
            # sin-product halves of both u-tiles share one tile so the
            # 32-row half-swap costs 4 DMAs per chunk instead of 8
            t1s = []
            w = wk_.tile([128, 2, SC], BF, name=f"rw_{c}", tag="rw", bufs=2)
            for u in range(2):
                pq = pp.tile([128, SC], F32, name=f"pq_{c}_{u}", tag="pq")
                for kb in range(KB):
                    nc.tensor.matmul(
                        pq,
                        lhsT=wq_sb[:, kb, u * 128:(u + 1) * 128],
                        rhs=xc[:, kb, :],
                        start=(kb == 0), stop=(kb == KB - 1),
                    )
                    if kb == 7:
                        # half-group yield: keeps units ~1.7us so scores/exp
                        # emission stays dense while a unit runs
                        yield
                t1 = wk_.tile([128, SC], BF, name=f"rt1_{c}_{u}", tag="rt",
                              bufs=3)
                nc.vector.tensor_mul(t1, pq, ctab_sb[:, cs])
                t1s.append(t1)
                nc.vector.tensor_mul(w[:, u, :], pq, stab_sb[:, cs])
                if u == 0:
                    yield
            wsw = wk_.tile([128, 2, SC], BF, name=f"wsw_{c}", tag="wsw",
                           bufs=2)
            for b in (0, 64):
                nc.sync.dma_start(out=wsw[b:b + 32, :, :],
                                  in_=w[b + 32:b + 64, :, :])
                nc.sync.dma_start(out=wsw[b + 32:b + 64, :, :],
                                  in_=w[b:b + 32, :, :])
            for u in range(2):
                nc.vector.tensor_add(qts[u][:, cs], t1s[u], wsw[:, u, :])
            yield
            # KV projection
            pkv = pp.tile([128, SC], F32, name=f"pkv_{c}", tag="pq")
            for kb in range(KB):
                nc.tensor.matmul(
                    pkv,
                    lhsT=wkv_sb[:, kb, :],
                    rhs=xc[:, kb, :],
                    start=(kb == 0), stop=(kb == KB - 1),
                )
                if kb == 7:
                    yield
            # K rope (rows 0-63)
            k1 = wk_.tile([64, SC], BF, name=f"kr1_{c}", tag="krt", bufs=2)
            nc.vector.tensor_mul(k1, pkv[0:64, :], ctab_sb[0:64, cs])
            kw = wk_.tile([64, SC], BF, name=f"krw_{c}", tag="krw", bufs=2)
            nc.vector.tensor_mul(kw, pkv[0:64, :], stab_sb[0:64, cs])
            ksw = wk_.tile([64, SC], BF, name=f"ksw_{c}", tag="ksw")
            nc.sync.dma_start(out=ksw[0:32, :], in_=kw[32:64, :])
            nc.sync.dma_start(out=ksw[32:64, :], in_=kw[0:32, :])
            nc.vector.tensor_add(kt_sb[0:64, cs], k1, ksw)
            nc.sync.dma_start(out=kt_sb[64:128, cs], in_=kt_sb[0:64, cs])
            # V natural layout: copy psum rows 64-127 (fp32), shift down via
            # DMA, then PE-transpose each 128-seq block
            vraw = wk_.tile([128, SC], F32, name=f"vraw_{c}", tag="vraw")
            nc.vector.tensor_copy(vraw[64:128, :], pkv[64:128, :])
            vtr = wk_.tile([64, SC], F32, name=f"vtr_{c}", tag="vtr")
            nc.sync.dma_start(out=vtr, in_=vraw[64:128, :])
            for r in range(4):
                j = 4 * c + r
                pt = pp.tile([128, HD], F32, name=f"pt_{c}_{r}", tag="ps")
                nc.tensor.transpose(pt, vtr[:, r * 128:(r + 1) * 128], eye_sb)
                nc.vector.tensor_copy(v_sb[:, j, 0:HD], pt)
                nc.vector.tensor_copy(v_sb2[:, j, HD:2 * HD], pt)
            yield

        def emit_attn(c, units):
            # `units` are ready-to-run generator steps (outproj of chunk c-1,
            # proj of chunk c+1) spread through the j-loop so the PE queue
            # always holds independent matmuls behind exp-gated ctx matmuls.
            cs = slice(c * SC, (c + 1) * SC)
            nslots = 2 * (4 * c + 4)
            quota, acc, ui = len(units) / nslots, 0.0, 0

            def fill():
                nonlocal acc, ui
                acc += quota
                while acc >= 1.0 and ui < len(units):
                    units[ui]()
                    ui += 1
                    acc -= 1.0

            for u in range(2):
                # idx0: lhsT=[ones|0*63|V] -> den row 0, ctx rows 64-127
                # idx1: lhsT=[V|ones|pad]  -> ctx rows 0-63, den row 64
                cps = [
                    pp.tile([128, SC], F32, name=f"cp_{c}_{u}_0", tag="pc"),
                    pp.tile([HD + 2, SC], F32, name=f"cp_{c}_{u}_1", tag="pc"),
                ]
                njt = 4 * c + 4
                for j in range(njt):
                    diag = j >= 4 * c
                    r = j - 4 * c
                    jb = slice(j * 128, (j + 1) * 128)
                    # columns of this chunk still unmasked for t-block j
                    lo = 128 * r if diag else 0
                    nsl = slice(lo, SC)
                    csl = slice(c * SC + lo, (c + 1) * SC)
                    for idx in range(2):
                        sp = pp.tile([128, SC], F32, name=f"sp_{c}_{u}_{j}_{idx}",
                                     tag="ps")
                        nc.tensor.matmul(
                            sp[:, nsl],
                            lhsT=kt_sb[idx * 64:idx * 64 + 64, jb],
                            rhs=qts[u][idx * 64:idx * 64 + 64, csl],
                            start=True, stop=True,
                            tile_position=(idx * 64, 0),
                        )
                        e = wk_.tile([128, SC], BF, name=f"e_{c}_{u}_{j}_{idx}",
                                     tag="ex", bufs=8)
                        nc.scalar.activation(e[:, nsl], sp[:, nsl], AF.Exp,
                                             scale=SCALE)
                        if diag:
                            dsl = slice(lo, lo + 128)
                            nc.vector.tensor_mul(e[:, dsl], e[:, dsl], tri_sb)
                        nc.tensor.matmul(
                            cps[idx][:, nsl],
                            lhsT=(v_sb2[:, j, :] if idx == 0
                                  else v_sb[:, j, :]),
                            rhs=e[:, nsl],
                            start=(j == 0), stop=(j == njt - 1),
                            skip_group_check=True,
                        )
                    fill()
                # idx0: den at psum row 0 -> reciprocal -> broadcast (from
                # partition 0) to all 128 partitions; ctx rows 64-127 scale
                # in place into cts rows 64-127. No cross-partition DMA.
                rec0 = wk_.tile([1, SC], F32, name=f"rec0_{c}_{u}",
                                tag="rec0", bufs=2)
                nc.vector.reciprocal_approx_fast(rec0, cps[0][0:1, :])
                cnv0 = wk_.tile([128, SC], BF, name=f"cnv0_{c}_{u}",
                                tag="cnv0", bufs=2)
                nc.vector.tensor_copy(cnv0[64:128, :], cps[0][64:128, :])
                bc0 = wk_.tile([128, SC], F32, name=f"bc0_{c}_{u}",
                               tag="bc0", bufs=2)
                nc.gpsimd.partition_broadcast(bc0, rec0)
                nc.vector.tensor_mul(cts[u][64:128, cs], cnv0[64:128, :],
                                     bc0[64:128, :])
                # idx1: den at psum row 64 -> copy to sbuf, DMA to partition
                # 0, reciprocal there, broadcast, scale rows 0-63.
                dcp = wk_.tile([66, SC], F32, name=f"dcp_{c}_{u}",
                               tag="dcp", bufs=2)
                nc.vector.tensor_copy(dcp[64:65, :], cps[1][64:65, :])
                den0 = wk_.tile([1, SC], F32, name=f"den0_{c}_{u}",
                                tag="den0", bufs=2)
                nc.sync.dma_start(out=den0, in_=dcp[64:65, :])
                rec1 = wk_.tile([1, SC], F32, name=f"rec1_{c}_{u}",
                                tag="rec1", bufs=2)
                nc.vector.reciprocal_approx_fast(rec1, den0)
                cnv1 = wk_.tile([64, SC], BF, name=f"cnv1_{c}_{u}",
                                tag="cnv1", bufs=2)
                nc.vector.tensor_copy(cnv1, cps[1][0:64, :])
                bc1 = wk_.tile([64, SC], F32, name=f"bc1_{c}_{u}",
                               tag="bc1", bufs=2)
                nc.gpsimd.partition_broadcast(bc1, rec1)
                nc.vector.tensor_mul(cts[u][0:64, cs], cnv1, bc1)

        def outproj_gen(c, tags=("po",)):
            """Out-projection for chunk c as 16 interleavable (mi, n) units.
            The final (drained) call rotates its psum tiles over all pool
            tags -- those banks are dead after the last attention chunk, and
            2 po banks alone make the tail matmuls wait on CAST evacuation."""
            for mi in range(4):
                m = 4 * c + mi
                mb = slice(m * 128, (m + 1) * 128)
                ob = wk_.tile([128, D], BF, name=f"ob_{c}_{mi}",
                              tag="ob", bufs=3)
                for n in range(4):
                    nck = slice(n * SC, (n + 1) * SC)
                    po = pp.tile([128, SC], F32, name=f"po_{c}_{mi}_{n}",
                                 tag=tags[(mi * 4 + n) % len(tags)])
                    for u in range(2):
                        nc.tensor.matmul(
                            po,
                            lhsT=cts[u][:, mb],
                            rhs=wo_sb[:, u, nck],
                            start=(u == 0), stop=(u == 1),
                        )
                    nc.vector.tensor_copy(ob[:, nck], po)
                    if n == 3:
                        if m >= 12:
                            # tail chunk: sync/scalar queues are idle by now;
                            # split across three queues so the final writes
                            # don't serialize on one DGE queue
                            nc.sync.dma_start(out=out[mb, 0:1024],
                                              in_=ob[:, 0:1024])
                            nc.scalar.dma_start(out=out[mb, 1024:2048],
                                                in_=ob[:, 1024:2048])
                        else:
                            nc.gpsimd.dma_start(out=out[mb, :], in_=ob)
                    yield

        def drain(gen):
            for _ in gen:
                pass

        def step(gen):
            return lambda: next(gen, None)

        def merge(a, b):
            # proportional interleave keeping each list's internal order
            res, ia, ib = [], 0, 0
            na, nb = len(a), len(b)
            while ia < na or ib < nb:
                if ib >= nb or (ia < na and ia * nb <= ib * na):
                    res.append(a[ia])
                    ia += 1
                else:
                    res.append(b[ib])
                    ib += 1
            return res

        NPU = 6  # units per proj_gen
        drain(proj_gen(0))
        for c in range(NSC):
            units = []
            if c >= 1:
                og = outproj_gen(c - 1)
                units = [step(og)] * 16
            if c + 1 < NSC:
                pg = proj_gen(c + 1)
                units = merge(units, [step(pg)] * NPU)
            emit_attn(c, units)
            # any leftovers (rounding) run here, before the next chunk
            if c >= 1:
                drain(og)
            if c + 1 < NSC:
                drain(pg)
        drain(outproj_gen(NSC - 1, tags=("po", "ps", "pc", "pq")))

    nc.finalize()
    return nc


def _get_nc():
    global _NC
    if _NC is None:
        _NC = _build()
    return _NC


def _prep_in_maps(x, Wq, Wk, Wv, Wo, cos, sin):
    BFNP = mybir.dt.np(BF)
    x0 = np.ascontiguousarray(np.asarray(x, np.float32).reshape(S, D))
    xT = np.ascontiguousarray(
        x0.T.reshape(KB, 128, S).transpose(1, 0, 2)).astype(BFNP)
    cosT = np.ascontiguousarray(np.asarray(cos, np.float32).T)  # (32, S)
    sinT = np.ascontiguousarray(np.asarray(sin, np.float32).T)
    ctab = np.tile(cosT, (4, 1)).astype(BFNP)                     # (128, S)
    # pre-swapped sign table: rope = q*C + swap32(q*S2)
    stab2 = np.tile(np.vstack([sinT, -sinT]), (2, 1)).astype(BFNP)
    trimask = (np.arange(128)[:, None] <= np.arange(128)[None, :]).astype(
        np.float32).astype(BFNP)
    eye = np.eye(64, dtype=np.float32)
    vones_a = np.zeros((128, KB, 2), np.float32)
    vones_a[:, :, 0] = 1.0
    vones_a = vones_a.astype(BFNP)
    vpad_a = np.zeros((128, KB, 63), np.float32).astype(BFNP)
    Wq = np.asarray(Wq, np.float32)
    Wk = np.asarray(Wk, np.float32)
    Wv = np.asarray(Wv, np.float32)
    Wo = np.asarray(Wo, np.float32)

    in_maps = []
    for i in range(NCORES):
        wq_i = np.ascontiguousarray(
            Wq[:, i * QC:(i + 1) * QC].reshape(KB, 128, QC).transpose(1, 0, 2)
        ).astype(BFNP)
        wkv_i = np.concatenate(
            [Wk[:, i * HD:(i + 1) * HD], Wv[:, i * HD:(i + 1) * HD]], axis=1)
        wkv_i = np.ascontiguousarray(
            wkv_i.reshape(KB, 128, 128).transpose(1, 0, 2)).astype(BFNP)
        wo_blocks = Wo[i * QC:(i + 1) * QC, :].reshape(2, 128, D)
        # cts rows 0-63 hold head 2u+1, rows 64-127 head 2u: swap halves
        wo_blocks = np.concatenate(
            [wo_blocks[:, 64:128, :], wo_blocks[:, 0:64, :]], axis=1)
        wo_i = np.ascontiguousarray(
            wo_blocks.transpose(1, 0, 2)).astype(BFNP)
        in_maps.append({
            "xT": xT, "wq": wq_i, "wkv": wkv_i, "wo": wo_i,
            "ctab": ctab, "stab2": stab2, "trimask": trimask, "eye": eye,
            "vones": vones_a, "vpad": vpad_a,
        })
    return in_maps


def run(inputs, **kw):
    nc = _get_nc()
    in_maps = _prep_in_maps(**inputs)
    return run_bass_kernel_spmd(nc, in_maps, list(range(NCORES)), **kw)


def kernel(x, Wq, Wk, Wv, Wo, cos, sin):
    res = run(dict(x=x, Wq=Wq, Wk=Wk, Wv=Wv, Wo=Wo, cos=cos, sin=sin))
    acc = np.zeros((S, D), np.float32)
    for r in res.results:
        acc += np.asarray(r["out"], np.float32)
    return acc.reshape(1, S, D)
